# revision 7
# baseline (speedup 1.0000x reference)
"""GPT-OSS attention QK+softmax block (sliding-window 128, softmax with sink)
for Trainium2, sharded over the 8 kv heads across 8 NeuronCores.

Reference computation (per kv head h, per q-head m):
    S = (q[:, h, m] @ k[:, h].T) / sqrt(64)            # [T, T]
    S += causal & sliding-window(128) mask             # band of width 128
    probs = softmax([S, sink_{h,m}])[..., :-1]         # sink column dropped

Device kernel design (v2):
  * band sparsity: only the two 128-wide key blocks (b-1, b) per query block b
    can be non-masked, so scores are [128, 256] tiles (128 for b=0).
  * fp16 matmul (fp32 PSUM accumulate): q*scale and k are O(+-6) with no
    dynamic-range needs, so fp16 keeps rel err ~2e-3, and PE streams 1
    col/cycle vs ~6 for fp32.
  * exp runs on raw (unmasked) scores, batched per m over the whole PSUM row
    ([128, 1792] + [128, 128]) -- no mask-add pass at all.  Masking happens in
    the DVE tensor_tensor_reduce: me = e * binmask with the row-sum accumulated
    in the same instruction, seeded with exp(sink) as the reduction's initial
    value, which yields the softmax denominator directly.
  * normalize-mul runs on the otherwise-idle gpsimd engine as
    scalar_tensor_tensor (InstTensorScalarPtr: gpsimd eff 0.6 vs 0.42 for
    plain tensor_tensor mult).
  * the DMA writes only the 128 valid columns per query row using diagonal
    access patterns (flat SBUF addressing: stride = row_pitch + 1), halving
    output bytes.  The masked-out region of the output is never written; the
    PJRT/NEFF execution path donates zero-initialized output buffers.
"""

import math
from contextlib import ExitStack

import numpy as np

T = 1024
HKV = 8
M = 8
D = 64
WINDOW = 128
NB = T // 128  # query blocks
SM_SCALE = 1.0 / math.sqrt(D)

_PROGRAM = None

# The DGE descriptor generator chunks the SBUF partition dim by 4 and
# recomputes each chunk's base from the canonical partition pitch, so flat
# "diagonal" APs (stride = row_pitch + 1) silently corrupt on hardware
# (CoreSim accepts them).  Keep the code path for reference, off by default.
DIAG_WRITES = False


def _build_program():
    import concourse.bacc as bacc
    import concourse.bass as bass
    import concourse.tile as tile
    from concourse import mybir

    f32 = mybir.dt.float32
    f16 = mybir.dt.float16
    Exp = mybir.ActivationFunctionType.Exp
    mult = mybir.AluOpType.mult
    add = mybir.AluOpType.add
    bypass = mybir.AluOpType.bypass

    nc = bacc.Bacc("TRN2")
    # qT: [D, M, T] bf16, pre-scaled by SM_SCALE.  kT: [D, T] bf16.
    qT = nc.dram_tensor("qT", [D, M, T], f16, kind="ExternalInput")
    kT = nc.dram_tensor("kT", [D, T], f16, kind="ExternalInput")
    sinks = nc.dram_tensor("sinks", [M], f32, kind="ExternalInput")
    # mask256[p, c] = 1.0 iff p+1 <= c <= p+128 (valid window within the
    # 256-wide two-block tile); tri128[p, c] = 1.0 iff c <= p.
    mask256 = nc.dram_tensor("mask256", [128, 256], f32, kind="ExternalInput")
    tri128 = nc.dram_tensor("tri128", [128, 128], f32, kind="ExternalInput")
    probs = nc.dram_tensor("probs", [M, T, T], f32, kind="ExternalOutput")

    W = 2048  # per-m row width in SBUF/PSUM: 8 blocks x 256 cols

    with tile.TileContext(nc) as tc, ExitStack() as ctx:
        singles = ctx.enter_context(tc.tile_pool(name="singles", bufs=1))
        psum_pool = ctx.enter_context(
            tc.tile_pool(name="psum", bufs=2, space="PSUM")
        )
        epool = ctx.enter_context(tc.tile_pool(name="epool", bufs=2))
        mepool = ctx.enter_context(tc.tile_pool(name="mepool", bufs=2))
        opool = ctx.enter_context(tc.tile_pool(name="opool", bufs=2))
        stats = ctx.enter_context(tc.tile_pool(name="stats", bufs=4))

        mask_sb = singles.tile([128, 256], f32)
        nc.sync.dma_start(out=mask_sb[:], in_=mask256[:])
        tri_sb = singles.tile([128, 128], f32)
        nc.sync.dma_start(out=tri_sb[:], in_=tri128[:])
        kT_sb = singles.tile([D, T], f16)
        nc.sync.dma_start(out=kT_sb[:], in_=kT[:])
        qT_sb = singles.tile([D, M, T], f16)
        nc.sync.dma_start(out=qT_sb[:], in_=qT[:])

        sink_bcast = bass.AP(tensor=sinks, offset=0, ap=[[0, 128], [1, M]])
        sink_sb = singles.tile([128, M], f32)
        nc.sync.dma_start(out=sink_sb[:], in_=sink_bcast)
        esink_sb = singles.tile([128, M], f32)
        nc.scalar.activation(out=esink_sb[:], in_=sink_sb[:], func=Exp)

        for m in range(M):
            ps = psum_pool.tile([128, W], f32)
            for b in range(NB):
                kw = 128 if b == 0 else 256
                koff = 0 if b == 0 else (b - 1) * 128
                nc.tensor.matmul(
                    ps[:, 256 * b : 256 * b + kw],
                    qT_sb[:, m, b * 128 : (b + 1) * 128],
                    kT_sb[:, koff : koff + kw],
                    start=True,
                    stop=True,
                )
            # exp of raw scores (cols 128:256 of the b=0 slot are never
            # written or read; skip them so no garbage flows through exp).
            e = epool.tile([128, W], f32)
            nc.scalar.activation(out=e[:, 0:128], in_=ps[:, 0:128], func=Exp)
            nc.scalar.activation(out=e[:, 256:W], in_=ps[:, 256:W], func=Exp)

            # masked exp + row-sum, fused in one standard DVE op per block:
            # me = (e * 1.0) * binmask, accum_out = rowsum(me).
            # (tensor_tensor_reduce would seed the sum with exp(sink) too,
            # but that custom DVE ISA op wedges the device at runtime.)
            me = mepool.tile([128, W], f32)
            rs = stats.tile([128, NB], f32)
            nc.vector.scalar_tensor_tensor(
                out=me[:, 0:128],
                in0=e[:, 0:128],
                scalar=1.0,
                in1=tri_sb[:],
                op0=mult,
                op1=mult,
                accum_out=rs[:, 0:1],
            )
            for b in range(1, NB):
                nc.vector.scalar_tensor_tensor(
                    out=me[:, 256 * b : 256 * (b + 1)],
                    in0=e[:, 256 * b : 256 * (b + 1)],
                    scalar=1.0,
                    in1=mask_sb[:],
                    op0=mult,
                    op1=mult,
                    accum_out=rs[:, b : b + 1],
                )
            den = stats.tile([128, NB], f32)
            nc.vector.tensor_scalar_add(den[:], rs[:], esink_sb[:, m : m + 1])
            rec = stats.tile([128, NB], f32)
            nc.vector.reciprocal(rec[:], den[:])

            # normalize: o_b = me_b * rec_b.  me is already masked (exact
            # zeros at invalid positions), so this holds for b=0 too.
            # Runs on the otherwise-idle gpsimd (Pool) engine; walrus
            # rejects scalar_tensor_tensor on Pool but takes tensor_scalar
            # with a per-partition AP scalar.
            o = opool.tile([128, W], f32)
            o_ap = o[:]
            nc.gpsimd.tensor_scalar_mul(o[:, 0:128], me[:, 0:128], rec[:, 0:1])
            for b in range(1, NB):
                nc.gpsimd.tensor_scalar_mul(
                    o[:, 256 * b : 256 * (b + 1)],
                    me[:, 256 * b : 256 * (b + 1)],
                    rec[:, b : b + 1],
                )

            # Output DMA: b=0 is a [128,128] rectangle at (0,0).
            nc.sync.dma_start(out=probs[m, 0:128, 0:128], in_=o[:, 0:128])
            if DIAG_WRITES:
                # Write only the 128 valid cols per row: source is a diagonal
                # AP over o (flat stride W+1), dest strides T+1 along rows.
                # Encoded 3D with explicit 4-partition chunks to match the
                # DGE descriptor generator's partition chunking.
                for b in range(1, NB):
                    src = bass.AP(
                        tensor=o_ap.tensor,
                        offset=o_ap.offset + 256 * b + 1,
                        ap=[[4 * (W + 1), 32], [W + 1, 4], [1, 128]],
                    )
                    dst = bass.AP(
                        tensor=probs,
                        offset=m * T * T + (128 * b) * T + 128 * (b - 1) + 1,
                        ap=[[4 * (T + 1), 32], [T + 1, 4], [1, 128]],
                    )
                    nc.sync.dma_start(out=dst, in_=src)
            else:
                # Banded 256-wide rows (exact zeros at masked positions).
                # Row-block stride in dest = 128*T + 128 elems.
                src = bass.AP(
                    tensor=o_ap.tensor,
                    offset=o_ap.offset + 256,
                    ap=[[W, 128], [256, NB - 1], [1, 256]],
                )
                dst = bass.AP(
                    tensor=probs,
                    offset=m * T * T + 128 * T,
                    ap=[[T, 128], [128 * T + 128, NB - 1], [1, 256]],
                )
                nc.sync.dma_start(out=dst, in_=src)

    nc.compile()
    return nc


def _get_program():
    global _PROGRAM
    if _PROGRAM is None:
        _PROGRAM = _build_program()
    return _PROGRAM


def _build_masks():
    il = np.arange(128)[:, None]
    jl = np.arange(256)[None, :]
    # valid window within the [b-1, b] two-block tile: p+1 <= c <= p+128
    mask256 = ((jl >= il + 1) & (jl <= il + 128)).astype(np.float32)
    jc = np.arange(128)[None, :]
    tri128 = (jc <= il).astype(np.float32)
    return mask256, tri128


def _make_in_maps(q, k, sinks):
    q = np.asarray(q, dtype=np.float32)
    k = np.asarray(k, dtype=np.float32)
    sinks = np.asarray(sinks, dtype=np.float32)
    mask256, tri128 = _build_masks()
    sinks_hm = sinks.reshape(HKV, M)
    in_maps = []
    for h in range(HKV):
        qT = np.ascontiguousarray(
            (q[:, h] * SM_SCALE).transpose(2, 1, 0)
        ).astype(np.float16)
        kT = np.ascontiguousarray(k[:, h].transpose(1, 0)).astype(np.float16)
        in_maps.append(
            {
                "qT": qT,
                "kT": kT,
                "sinks": np.ascontiguousarray(sinks_hm[h]),
                "mask256": mask256,
                "tri128": tri128,
            }
        )
    return in_maps


def _run(q, k, sinks, trace=False):
    from concourse.bass_utils import run_bass_kernel_spmd

    nc = _get_program()
    in_maps = _make_in_maps(q, k, sinks)
    res = run_bass_kernel_spmd(nc, in_maps, list(range(HKV)), trace=trace)
    out = np.stack([r["probs"] for r in res.results], axis=0)
    return out, res


def kernel(q, k, sinks):
    out, _ = _run(q, k, sinks, trace=False)
    return out
